# revision 39
# baseline (speedup 1.0000x reference)
"""Trainium2 Bass kernel for nn_CCR_59193239273568 (3-pass spatial attention block).

v2 strategy (8 NeuronCores, SPMD), all-bf16 compute (rel err ~4e-3 < 2e-2):
  - Phase A: per-core 8-row band of both samples. q/k/v conv1 fused via a
    96-wide stationary (wq1|wk1|wv1), conv2 via a block-diagonal [96,96]
    stationary: 3 streams for the cost of one moving pass. Only the straight
    [C, px] bands are AllGathered (bf16, one collective per sample); each
    receiver rebuilds the transposed V^T|ones form locally with PE
    transposes right before that sample's attention units.
  - Phase B: per (sample, pass): S^T chunks [128 keys, 512 queries] as 32
    bf16 matmuls (GS=2 PSUM groups), exp on ScalarE straight out of PSUM
    (scale folded into the activation), bf16 exp output. ctx^T + row-sum
    accumulate as one [33, 512] PSUM group with a single V^T|ones
    stationary; normalization via DVE reciprocal + gpsimd
    partition_broadcast.
  - Phase C: only the 3-row ctx halos cross cores: a tiny per-sample edge
    AllGather, read back with host-provided wraparound neighbor indices
    (register ds), image borders zeroed by a per-core host mask. Fully
    local convs: wr/wg/wb fused block-diagonal (halo-free middle rows
    split out so they run before the edge gather lands), the 3-way average
    as a PE matmul with a stacked-identity stationary (1/3 folded into
    w2), w2 on the concatenated [96,...] picture, w3, output band.
    SAME-padding edges handled with host-built bias images (-1e30 rows
    relu to zero).
  - Scheduling: artificial zero-valued "token" ops pin the s1 transpose
    block and the phase-C conv chains so the legacy tile scheduler cannot
    hoist them into stall points of the in-order PE queue.
"""

import sys

import numpy as np
import ml_dtypes

sys.path.insert(0, "/opt/trn_rl_repo")

import concourse.bacc as bacc
import concourse.bass as bass
import concourse.mybir as mybir
import concourse.tile as tile
from concourse.bass_utils import run_bass_kernel_spmd

F32 = mybir.dt.float32
BF16 = mybir.dt.bfloat16
AF = mybir.ActivationFunctionType
ALU = mybir.AluOpType
NPBF16 = ml_dtypes.bfloat16

B, CIN, C, H, W = 2, 64, 32, 64, 64
R = 8                 # cores
BR = H // R           # 8 band rows per core per sample
PX = BR * W           # 512 band pixels
N = H * W             # 4096
SCALE = float(C) ** -0.5
NCH = N // 128        # 32 key chunks per sample
GS = 2                # exp group size in chunks (PSUM: 2x2 banks for S)

STR_SZ = 3 * C * PX           # 49152  straight-band gather payload per sample
E_SZ = 3 * 2 * C * 3 * W      # 36864  per-sample ctx edge contribution
ROWB = 3 * W                  # 192 elems per 3-row edge


def build_program():
    nc = bacc.Bacc("TRN2", target_bir_lowering=False, debug=False, num_devices=R)

    xband_d = nc.declare_dram_parameter("xband", [CIN, B, 12, 66], BF16, isOutput=False)
    w1_d = nc.declare_dram_parameter("w1", [CIN, 9, 96], BF16, isOutput=False)
    wqkv2_d = nc.declare_dram_parameter("wqkv2", [96, 9, 96], BF16, isOutput=False)
    wrgb_d = nc.declare_dram_parameter("wrgb", [96, 9, 96], BF16, isOutput=False)
    w2_d = nc.declare_dram_parameter("w2", [96, 9, C], BF16, isOutput=False)
    w3_d = nc.declare_dram_parameter("w3", [C, 9, C], BF16, isOutput=False)
    biasA_d = nc.declare_dram_parameter("biasA", [96, 10, W], F32, isOutput=False)
    biasC_d = nc.declare_dram_parameter("biasC", [96, 12, W], F32, isOutput=False)
    biasD_d = nc.declare_dram_parameter("biasD", [C, 10, W], F32, isOutput=False)
    bqkv2_d = nc.declare_dram_parameter("bqkv2", [96, 1], F32, isOutput=False)
    b3_d = nc.declare_dram_parameter("b3", [C, 1], F32, isOutput=False)
    nbr_d = nc.declare_dram_parameter("nbr", [1, 2], mybir.dt.uint32, isOutput=False)
    hmask_d = nc.declare_dram_parameter("hmask", [96, 2], F32, isOutput=False)
    id3x_d = nc.declare_dram_parameter("id3x", [96, 32], BF16, isOutput=False)
    id32_d = nc.declare_dram_parameter("id32", [32, 32], BF16, isOutput=False)
    out_d = nc.declare_dram_parameter("out", [B, C, BR, W], F32, isOutput=True)

    rg = [list(range(R))]

    with tile.TileContext(nc) as tc:
        with (
            tc.tile_pool(name="const", bufs=1) as constp,
            tc.tile_pool(name="persist", bufs=1) as persistp,
            tc.tile_pool(name="exp", bufs=3) as ep,
            tc.tile_pool(name="small", bufs=2) as smallp,
            tc.tile_pool(name="phc", bufs=2) as phcp,
            tc.tile_pool(name="ps_conv", bufs=2, space="PSUM") as ps_conv,
            tc.tile_pool(name="ps_s", bufs=2, space="PSUM") as ps_s,
            tc.tile_pool(name="ps_ctx", bufs=2, space="PSUM") as ps_ctx,
            tc.tile_pool(name="dram", bufs=1, space="DRAM") as dramp,
        ):
            prevreg = nc.sync.alloc_register("prev_slot")
            nc.sync.reg_load(prevreg, nbr_d[0:1, 0:1])
            prv = nc.sync.snap(prevreg, donate=True, min_val=0, max_val=R - 1)
            nxtreg = nc.sync.alloc_register("next_slot")
            nc.sync.reg_load(nxtreg, nbr_d[0:1, 1:2])
            nxt = nc.sync.snap(nxtreg, donate=True, min_val=0, max_val=R - 1)

            # ---------------- constants into SBUF ----------------
            # phase-A-critical inputs first: the SP DMA queue is in-order
            xband_sb = constp.tile([CIN, B, 12, 66], BF16, tag="xband")
            nc.sync.dma_start(xband_sb[:, 0, :, :], xband_d[:, 0, :, :])
            w1_sb = constp.tile([CIN, 9, 96], BF16, tag="w1")
            nc.sync.dma_start(w1_sb[:], w1_d[:])
            biasA_sb = constp.tile([96, 10, W], F32, tag="biasA")
            nc.sync.dma_start(biasA_sb[:], biasA_d[:])
            wqkv2_sb = constp.tile([96, 9, 96], BF16, tag="wqkv2")
            nc.sync.dma_start(wqkv2_sb[:], wqkv2_d[:])
            bqkv2_sb = constp.tile([96, 1], F32, tag="bqkv2")
            nc.sync.dma_start(bqkv2_sb[:], bqkv2_d[:])
            id32_sb = constp.tile([32, 32], BF16, tag="id32")
            nc.sync.dma_start(id32_sb[:], id32_d[:])
            nc.sync.dma_start(xband_sb[:, 1, :, :], xband_d[:, 1, :, :])
            wrgb_sb = constp.tile([96, 9, 96], BF16, tag="wrgb")
            nc.sync.dma_start(wrgb_sb[:], wrgb_d[:])
            w2_sb = constp.tile([96, 9, C], BF16, tag="w2")
            nc.sync.dma_start(w2_sb[:], w2_d[:])
            w3_sb = constp.tile([C, 9, C], BF16, tag="w3")
            nc.sync.dma_start(w3_sb[:], w3_d[:])
            biasC_sb = constp.tile([96, 12, W], F32, tag="biasC")
            nc.sync.dma_start(biasC_sb[:], biasC_d[:])
            biasD_sb = constp.tile([C, 10, W], F32, tag="biasD")
            nc.sync.dma_start(biasD_sb[:], biasD_d[:])
            b3_sb = constp.tile([C, 1], F32, tag="b3")
            nc.sync.dma_start(b3_sb[:], b3_d[:])
            hmask_sb = constp.tile([96, 2], F32, tag="hmask")
            nc.sync.dma_start(hmask_sb[:], hmask_d[:])
            id3x_sb = constp.tile([96, 32], BF16, tag="id3x")
            nc.sync.dma_start(id3x_sb[:], id3x_d[:])

            # warm the exp table early (overlaps with phase A)
            dummy = constp.tile([1, 16], F32, tag="dummy")
            nc.vector.memset(dummy[:], 0.0)
            nc.scalar.activation(dummy[:], dummy[:], AF.Exp)
            ones1_sb = constp.tile([1, C], F32, tag="ones1")
            nc.vector.memset(ones1_sb[:], 1.0)

            # ---------------- collective buffers ----------------
            contrib1 = [
                dramp.tile([STR_SZ], BF16, tag=f"c1_{s}", name=f"contrib1_{s}")
                for s in range(B)
            ]
            gath1a = [
                dramp.tile([R, 2 * C * PX], BF16, tag=f"g1a_{s}",
                           name=f"gath1a_{s}", addr_space="Shared")
                for s in range(B)
            ]
            gath1b = [
                dramp.tile([R, C * PX], BF16, tag=f"g1b_{s}",
                           name=f"gath1b_{s}", addr_space="Shared")
                for s in range(B)
            ]
            contrib2 = [
                dramp.tile([E_SZ], BF16, tag=f"c2_{s}", name=f"contrib2_{s}")
                for s in range(B)
            ]
            gath2 = [
                dramp.tile([R, E_SZ], BF16, tag=f"g2_{s}", name=f"gath2_{s}",
                           addr_space="Shared")
                for s in range(B)
            ]

            def relu_img(out_ap, psum_ap, bimg_ap, tmp_shape, tmp_tag):
                # out = max(psum + bias_image, 0); -1e30 rows relu to zero
                tmpb = smallp.tile(tmp_shape, F32, tag=tmp_tag, name="tmpb")
                nc.vector.tensor_add(tmpb[:], psum_ap, bimg_ap)
                nc.vector.tensor_scalar(out_ap, tmpb[:], 0.0, None, ALU.max)

            # ---------------- phase A: q/k/v bands ----------------
            qb = {}       # (s, t) -> [32, 8, 64] bf16 band
            for s in range(B):
                q1pad = persistp.tile([96, 10, 66], BF16, tag=f"q1pad_{s}")
                nc.vector.memset(q1pad[:, :, 0:1], 0.0)
                nc.vector.memset(q1pad[:, :, 65:66], 0.0)
                for j0 in (0, 5):
                    ps = ps_conv.tile([96, 5, W], F32, tag="c", name="cps")
                    for tap in range(9):
                        dy, dx = divmod(tap, 3)
                        nc.tensor.matmul(
                            ps[:],
                            w1_sb[:, tap, :],
                            xband_sb[:, s, j0 + dy:j0 + dy + 5, dx:dx + W],
                            start=(tap == 0), stop=(tap == 8),
                        )
                    relu_img(q1pad[:, j0:j0 + 5, 1:65], ps[:],
                             biasA_sb[:, j0:j0 + 5, :], [96, 5, W], "tA")

                ps2 = ps_conv.tile([96, BR, W], F32, tag="c", name="cps2")
                for tap in range(9):
                    dy, dx = divmod(tap, 3)
                    nc.tensor.matmul(
                        ps2[:],
                        wqkv2_sb[:, tap, :],
                        q1pad[:, dy:dy + BR, dx:dx + W],
                        start=(tap == 0), stop=(tap == 8),
                    )
                for t in range(3):
                    qt = persistp.tile([C, BR, W], BF16, tag=f"qb_{s}_{t}")
                    nc.vector.tensor_scalar(
                        qt[:], ps2[32 * t:32 * t + 32, :, :],
                        bqkv2_sb[32 * t:32 * t + 32, :], 0.0,
                        ALU.add, ALU.max,
                    )
                    qb[(s, t)] = qt[:]

                for t in (1, 2, 0):  # k+v first: their gather must fire first
                    nc.sync.dma_start(
                        contrib1[s][t * C * PX:(t + 1) * C * PX]
                        .rearrange("(c px) -> c px", c=C),
                        qb[(s, t)].rearrange("c a w -> c (a w)"),
                    )
                nc.gpsimd.collective_compute(
                    "AllGather", ALU.bypass, replica_groups=rg,
                    ins=[contrib1[s][0:C * PX]], outs=[gath1b[s][:]],
                )
                nc.gpsimd.collective_compute(
                    "AllGather", ALU.bypass, replica_groups=rg,
                    ins=[contrib1[s][C * PX:3 * C * PX]],
                    outs=[gath1a[s][:]],
                )

            # ---------------- gathered reloads ----------------
            # SP DMA queue is in-order: k+v(s0), q(s0), k+v(s1) land here;
            # q(s1) is emitted at the top of s1's unit block so the s0 edge
            # DMAs are not head-of-line blocked behind its gather wait
            ks_all = []   # [32, 3 t, 8 g, 512] bf16 per sample
            for s in range(B):
                ks = persistp.tile([C, 3, R, PX], BF16, tag=f"ks_{s}")
                for t in (1, 2):
                    nc.sync.dma_start(
                        ks[:, t, :, :],
                        gath1a[s][:, (t - 1) * C * PX:t * C * PX]
                        .rearrange("g (c px) -> c g px", c=C),
                    )
                if s == 0:
                    nc.sync.dma_start(
                        ks[:, 0, :, :],
                        gath1b[s][:].rearrange("g (c px) -> c g px", c=C),
                    )
                ks_all.append(ks)

            # ---------------- phase B: attention units ----------------
            ctxband = {}  # (s, p) -> [32, 512] bf16 normalized ctx rows
            # token-gated identity copies: the tile scheduler may not hoist
            # a transpose block (stalling the in-order PE queue on a pending
            # gather) before the unit its identity is chained behind
            idkv1 = constp.tile([32, 32], BF16, tag="idkv1")
            idq0 = constp.tile([32, 32], BF16, tag="idq0")
            idq1 = constp.tile([32, 32], BF16, tag="idq1")
            idq = [idq0, idq1]
            tokC = constp.tile([C, 1], F32, tag="tokC")

            def release_ident(dst, cb_ap):
                tokz = smallp.tile([C, 1], F32, tag="tokz")
                nc.vector.tensor_scalar(tokz[:], cb_ap, 0.0, None, ALU.mult)
                nc.vector.tensor_scalar(dst[:], id32_sb[:], tokz[:],
                                        None, ALU.add)

            for s in range(B):
                ks = ks_all[s]
                if s == 1:
                    nc.sync.dma_start(
                        ks[:, 0, :, :],
                        gath1b[s][:].rearrange("g (c px) -> c g px", c=C),
                    )
                vta = persistp.tile([128, R, 4, 3, 33], BF16, tag=f"vta_{s}")
                nc.vector.memset(vta[:, :, :, :, 32:33], 1.0)
                identkv = idkv1 if s == 1 else id32_sb
                for g in range(R):
                    trp = ps_conv.tile([128, 4, 2, 32], BF16, tag="c",
                                       name="trp")
                    for ii in range(4):
                        for t in (1, 2):
                            nc.tensor.transpose(
                                trp[:, ii, t - 1, :],
                                ks[:, t, g, 128 * ii:128 * ii + 128],
                                identkv[:],
                            )
                    nc.vector.tensor_copy(vta[:, g, :, 1:3, 0:32], trp[:])
                vtl_s = vta.rearrange("p g a t c -> p g (a t c)")
                for p in range(3):
                    if p == 1:
                        # q-stream transposes: needed first by this unit's
                        # ctx matmuls, data from the later q gather
                        for g in range(R):
                            trq = ps_conv.tile([128, 4, 32], BF16, tag="c",
                                               name="trq")
                            for ii in range(4):
                                nc.tensor.transpose(
                                    trq[:, ii, :],
                                    ks[:, 0, g, 128 * ii:128 * ii + 128],
                                    idq[s][:],
                                )
                            nc.vector.tensor_copy(vta[:, g, :, 0, 0:32],
                                                  trq[:])
                    qt, kt, vt = p, (p + 1) % 3, (p + 2) % 3
                    qrhs = qb[(s, qt)].rearrange("c a w -> c (a w)")
                    ksl = ks_all[s]
                    vtl = vtl_s

                    # ctx accumulated as [33, 512]: rows 0:32 = ctx^T, row 32
                    # = exp row-sum (ones column folded into the V^T pack);
                    # one PSUM accumulation group per unit (one zero region)
                    ctxps = ps_ctx.tile([33, PX], F32, tag="ctx")
                    ngroups = NCH // GS

                    def emit_s_group(g):
                        sps = ps_s.tile([128, GS * PX], F32, tag="s", name="sps")
                        for ci in range(GS):
                            i = g * GS + ci
                            rr, ip = divmod(i, 4)
                            nc.tensor.matmul(
                                sps[:, ci * PX:(ci + 1) * PX],
                                ksl[:, kt, rr, 128 * ip:128 * ip + 128],
                                qrhs,
                                start=True, stop=True,
                            )
                        return sps

                    def emit_ctx_group(g, es):
                        for ci in range(GS):
                            i = g * GS + ci
                            rr, ip = divmod(i, 4)
                            off = ip * 99 + vt * 33
                            nc.tensor.matmul(
                                ctxps[:],
                                vtl[:, rr, off:off + 33],
                                es[:, ci * PX:(ci + 1) * PX],
                                start=(i == 0), stop=(i == NCH - 1),
                            )

                    # software pipeline: S(g+1) emitted before ctx(g)
                    sps = emit_s_group(0)
                    for g in range(ngroups):
                        es = ep.tile([128, GS * PX], BF16, tag="e")
                        nc.scalar.activation(es[:], sps[:], AF.Exp, scale=SCALE)
                        if g + 1 < ngroups:
                            sps = emit_s_group(g + 1)
                        emit_ctx_group(g, es)

                    rs = smallp.tile([1, PX], F32, tag="rs")
                    nc.vector.tensor_copy(rs[:], ctxps[32:33, :])
                    recip = smallp.tile([1, PX], F32, tag="recip")
                    nc.vector.reciprocal(recip[:], rs[:])
                    bcast = smallp.tile([C, PX], F32, tag="bcast")
                    nc.gpsimd.partition_broadcast(bcast[:], recip[:])
                    cb = persistp.tile([C, PX], BF16, tag=f"cb_{s}_{p}")
                    nc.vector.tensor_mul(cb[:], ctxps[0:32, :], bcast[:])
                    ctxband[(s, p)] = cb
                    if p == 0:
                        release_ident(idq[s], cb[:, 0:1])
                    if (s, p) == (0, 2):
                        release_ident(idkv1, cb[:, 0:1])
                    if (s, p) == (1, 2):
                        nc.vector.tensor_scalar(tokC[:], cb[:, 0:1], 0.0,
                                                None, ALU.mult)

                    for e in range(2):
                        off = (p * 2 + e) * C * ROWB
                        src = cb[:, 0:ROWB] if e == 0 else cb[:, PX - ROWB:PX]
                        nc.sync.dma_start(
                            contrib2[s][off:off + C * ROWB]
                            .rearrange("(c w) -> c w", c=C),
                            src,
                        )

                nc.gpsimd.collective_compute(
                    "AllGather", ALU.bypass, replica_groups=rg,
                    ins=[contrib2[s][:]], outs=[gath2[s][:]],
                )

            # ---------------- phase C: output convs ----------------
            for s in range(B):
                ctx3 = phcp.tile([96, 14, 66], BF16, tag="ctx3")
                ctxmid = phcp.tile([96, 8, 66], BF16, tag="ctxmid")
                nc.vector.memset(ctx3[:, :, 0:1], 0.0)
                nc.vector.memset(ctx3[:, :, 65:66], 0.0)
                nc.vector.memset(ctxmid[:, :, 0:1], 0.0)
                nc.vector.memset(ctxmid[:, :, 65:66], 0.0)
                # token dep: keep phase-C convs out of the PE queue until all
                # attention units have drained (writes a guard zero)
                nc.vector.tensor_scalar(ctx3[0:32, 0, 0:1], tokC[:], 0.0,
                                        None, ALU.mult)
                nc.vector.tensor_scalar(ctxmid[0:32, 0, 0:1], tokC[:], 0.0,
                                        None, ALU.mult)
                for p in range(3):
                    band = ctxband[(s, p)][:].rearrange("c (a w) -> c a w", w=W)
                    nc.vector.tensor_copy(
                        ctx3[32 * p:32 * p + 32, 3:11, 1:65], band)
                    nc.vector.tensor_copy(
                        ctxmid[32 * p:32 * p + 32, :, 1:65], band)
                # halo rows straight from the edge gather: previous rank's
                # bottom edge / next rank's top edge (wraparound indices from
                # the host; image borders multiplied to zero by hmask)
                g2v = gath2[s].rearrange("g (p e c a w) -> c (g p e) a w",
                                         p=3, e=2, c=C, a=3)
                sct = phcp.tile([96, 3, W], BF16, tag="sct")
                scb = phcp.tile([96, 3, W], BF16, tag="scb")
                for p in range(3):
                    nc.sync.dma_start(
                        sct[32 * p:32 * p + 32, :, :],
                        g2v[:, bass.ds(prv * 6 + 2 * p + 1, 1), :, :],
                    )
                    nc.sync.dma_start(
                        scb[32 * p:32 * p + 32, :, :],
                        g2v[:, bass.ds(nxt * 6 + 2 * p, 1), :, :],
                    )
                nc.vector.tensor_scalar(ctx3[:, 0:3, 1:65], sct[:],
                                        hmask_sb[:, 0:1], None, ALU.mult)
                nc.vector.tensor_scalar(ctx3[:, 11:14, 1:65], scb[:],
                                        hmask_sb[:, 1:2], None, ALU.mult)

                # wr/wg/wb fused conv -> tmp [96, 12, 64] bf16; the middle
                # rows touch no halo, so that group runs before the edge
                # exchange lands and only the two 3-row edge groups sit in
                # the post-gather tail
                tmp = phcp.tile([96, 12, W], BF16, tag="tmp")
                for j0, nr in ((3, 6), (0, 3), (9, 3)):
                    ps = ps_conv.tile([96, nr, W], F32, tag="c", name="cps3")
                    for tap in range(9):
                        dy, dx = divmod(tap, 3)
                        if j0 == 3:  # halo-free middle rows read ctxmid
                            mv = ctxmid[:, dy:dy + nr, dx:dx + W]
                        else:
                            mv = ctx3[:, j0 + dy:j0 + dy + nr, dx:dx + W]
                        nc.tensor.matmul(
                            ps[:],
                            wrgb_sb[:, tap, :],
                            mv,
                            start=(tap == 0), stop=(tap == 8),
                        )
                    relu_img(tmp[:, j0:j0 + nr, :], ps[:],
                             biasC_sb[:, j0:j0 + nr, :], [96, nr, W], "tC")

                # x | (ctx1+ctx2+ctx3) picture (1/3 folded into w2); the
                # 3-way sum across partition blocks is a PE matmul with a
                # stacked-identity stationary (partition-offset tensor adds
                # are rejected by the BIR verifier)
                xctx = phcp.tile([96, 12, 66], BF16, tag="xctx")
                nc.vector.memset(xctx[64:96, :, 0:1], 0.0)
                nc.vector.memset(xctx[64:96, :, 65:66], 0.0)
                nc.vector.tensor_copy(xctx[0:64, :, :], xband_sb[:, s, :, :])
                avgps = ps_s.tile([C, 12, W], F32, tag="s", name="avgps")
                for r0, r1 in ((0, 8), (8, 12)):  # stay within one PSUM bank
                    nc.tensor.matmul(
                        avgps[:, r0:r1, :], id3x_sb[:], tmp[:, r0:r1, :],
                        start=True, stop=True,
                    )
                nc.vector.tensor_copy(xctx[64:96, :, 1:65], avgps[:])

                w2pad = phcp.tile([C, 10, 66], BF16, tag="w2pad")
                nc.vector.memset(w2pad[:, :, 0:1], 0.0)
                nc.vector.memset(w2pad[:, :, 65:66], 0.0)
                for j0 in (0, 5):
                    ps = ps_conv.tile([C, 5, W], F32, tag="c", name="cps4")
                    for tap in range(9):
                        dy, dx = divmod(tap, 3)
                        nc.tensor.matmul(
                            ps[:],
                            w2_sb[:, tap, :],
                            xctx[:, j0 + dy:j0 + dy + 5, dx:dx + W],
                            start=(tap == 0), stop=(tap == 8),
                        )
                    relu_img(w2pad[:, j0:j0 + 5, 1:65], ps[:],
                             biasD_sb[:, j0:j0 + 5, :], [C, 5, W], "tD")

                ps = ps_conv.tile([C, BR, W], F32, tag="c", name="cps5")
                for tap in range(9):
                    dy, dx = divmod(tap, 3)
                    nc.tensor.matmul(
                        ps[:],
                        w3_sb[:, tap, :],
                        w2pad[:, dy:dy + BR, dx:dx + W],
                        start=(tap == 0), stop=(tap == 8),
                    )
                outsb = smallp.tile([C, BR, W], F32, tag="outsb")
                nc.vector.tensor_scalar(outsb[:], ps[:], b3_sb[:], 0.0,
                                        ALU.add, ALU.max)
                nc.sync.dma_start(out_d[s], outsb[:])

    nc.compile()
    return nc


def _pack_w(w):
    # [Cout, Cin, 3, 3] -> lhsT pack [Cin, 9, Cout] bf16
    w = np.asarray(w, np.float32)
    return np.ascontiguousarray(
        w.transpose(1, 2, 3, 0).reshape(w.shape[1], 9, w.shape[0])
    ).astype(NPBF16)


NEG = np.float32(-1e30)


def prep_in_maps(inputs):
    x = np.asarray(inputs["x"], np.float32)
    xp = np.zeros((B, CIN, H + 4, W + 2), np.float32)
    xp[:, :, 2:2 + H, 1:1 + W] = x

    f32 = lambda k: np.asarray(inputs[k], np.float32)

    shared = {}
    shared["w1"] = np.concatenate(
        [_pack_w(inputs["wq1"]), _pack_w(inputs["wk1"]), _pack_w(inputs["wv1"])],
        axis=2,
    )
    for name, keys in (("wqkv2", ("wq2", "wk2", "wv2")),
                       ("wrgb", ("wr", "wg", "wb"))):
        blk = np.zeros((96, 9, 96), np.float32)
        for t, k in enumerate(keys):
            blk[32 * t:32 * t + 32, :, 32 * t:32 * t + 32] = _pack_w(inputs[k])
        shared[name] = blk.astype(NPBF16)
    w2 = f32("w2").copy()
    w2[:, CIN:, :, :] /= 3.0   # fold the ctx 3-way average into w2
    shared["w2"] = _pack_w(w2)
    shared["w3"] = _pack_w(inputs["w3"])
    shared["bqkv2"] = np.ascontiguousarray(
        np.concatenate([f32("bq2"), f32("bk2"), f32("bv2")])[:, None]
    )
    shared["b3"] = np.ascontiguousarray(f32("b3")[:, None])
    shared["id32"] = np.eye(32, dtype=NPBF16)
    shared["id3x"] = np.tile(np.eye(32, dtype=np.float32), (3, 1)).astype(NPBF16)

    bA = np.concatenate(
        [np.broadcast_to(f32(bn)[:, None, None], (C, 10, W)) for bn in
         ("bq1", "bk1", "bv1")], axis=0).copy()     # [96, 10, W]; rows r0-1..r0+8
    bC = np.concatenate(
        [np.broadcast_to(f32(bn)[:, None, None], (C, 12, W)) for bn in
         ("br", "bg", "bb")], axis=0).copy()        # [96, 12, W]; rows r0-2..r0+9
    bD = np.broadcast_to(f32("b2")[:, None, None], (C, 10, W)).copy()

    in_maps = []
    for r in range(R):
        r0 = BR * r
        xband = np.ascontiguousarray(
            xp[:, :, r0:r0 + 12, :].transpose(1, 0, 2, 3)
        ).astype(NPBF16)  # [CIN, B, 12, 66], rows r0-2..r0+9

        biasA, biasC, biasD = bA.copy(), bC.copy(), bD.copy()
        if r == 0:
            biasA[:, 0, :] = NEG
            biasC[:, 0:2, :] = NEG
            biasD[:, 0, :] = NEG
        if r == R - 1:
            biasA[:, 9, :] = NEG
            biasC[:, 10:12, :] = NEG
            biasD[:, 9, :] = NEG

        hmask = np.ones((96, 2), np.float32)
        if r == 0:
            hmask[:, 0] = 0.0
        if r == R - 1:
            hmask[:, 1] = 0.0
        in_maps.append(dict(
            shared, xband=xband,
            nbr=np.array([[(r - 1) % R, (r + 1) % R]], np.uint32),
            hmask=hmask,
            biasA=np.ascontiguousarray(biasA),
            biasC=np.ascontiguousarray(biasC),
            biasD=np.ascontiguousarray(biasD),
        ))
    return in_maps


_CACHE = {}


def get_program():
    if "nc" not in _CACHE:
        _CACHE["nc"] = build_program()
    return _CACHE["nc"]


def kernel(**inputs):
    nc = get_program()
    in_maps = prep_in_maps(inputs)
    res = run_bass_kernel_spmd(nc, in_maps, list(range(R)))
    out = np.zeros((B, C, H, W), np.float32)
    for r in range(R):
        out[:, :, BR * r:BR * (r + 1), :] = res.results[r]["out"]
    return out
